# revision 5
# baseline (speedup 1.0000x reference)
"""Multi-head attention (RoPE + softmax) forward for Trainium2, 8 NeuronCores.

Problem: B=4, S=2048, D=2048, H=16 heads (hd=128), fp32 in/out.

Sharding: core c handles batch b = c//2 and head-group g = c%2 (8 heads).
Each core computes QKV projections for its batch restricted to its head
group, rotary, attention, and a partial output projection (contraction
over its 1024 features of wo).  The two partials per batch are summed on
the host.

Layout tricks (all host-side prep, free):
  - x is fed transposed (xT [D, S]) so it serves directly as matmul
    operands for both transposed (q/k) and natural (v) projections.
  - wq/wk columns are permuted per head so rotary pairs land in
    half-layout (real parts rows 0:64, imag rows 64:128 of each head
    block).  Rotary then works on contiguous partition halves.
  - mask folds into the softmax exp as a per-partition bias (scores are
    computed k-major: [k partitions, q free]).
  - softmax skips max-subtraction (inputs are well-scaled gaussians;
    scores are bounded ~|7|, exp stays in fp32 range).

All matmuls run as float32r (full PE rate at N=512, TF32-ish accuracy).
"""

import math

import numpy as np

B, S, D = 4, 2048, 2048
H_PER_CORE = 8  # heads per core
HD = 128  # head dim
F = 1024  # features per core (head group)
P = 128
DT = D // P  # 16 contraction tiles
NCORES = 8
SCALE = 1.0 / math.sqrt(HD)

_CACHE = {}


def _build():
    import concourse.bacc as bacc
    import concourse.mybir as mybir
    import concourse.tile as tile

    f32 = mybir.dt.float32
    f32r = mybir.dt.float32r
    EXP = mybir.ActivationFunctionType.Exp

    nc = bacc.Bacc("TRN2", target_bir_lowering=False, debug=False, num_devices=NCORES)

    xT = nc.dram_tensor("xT", [D, S], f32r, kind="ExternalInput")
    wq = nc.dram_tensor("wq", [D, F], f32r, kind="ExternalInput")
    wk = nc.dram_tensor("wk", [D, F], f32r, kind="ExternalInput")
    wv = nc.dram_tensor("wv", [D, F], f32r, kind="ExternalInput")
    wo = nc.dram_tensor("wo", [F, D], f32r, kind="ExternalInput")
    cosT_d = nc.dram_tensor("cosT", [P, S], f32, kind="ExternalInput")
    sinT_d = nc.dram_tensor("sinT", [P, S], f32, kind="ExternalInput")
    mask_d = nc.dram_tensor("maskT", [P, DT], f32, kind="ExternalInput")
    ones_k_d = nc.dram_tensor("ones_k", [P, 1], f32r, kind="ExternalInput")
    ones_p_d = nc.dram_tensor("ones_p", [1, P], f32r, kind="ExternalInput")
    out_d = nc.dram_tensor("out", [S, D], f32, kind="ExternalOutput")

    qT_d = nc.dram_tensor("qT_scratch", [F, S], f32r, kind="Internal")
    kT_d = nc.dram_tensor("kT_scratch", [F, S], f32r, kind="Internal")
    v_d = nc.dram_tensor("v_scratch", [S, F], f32r, kind="Internal")

    with tile.TileContext(nc) as tc, nc.allow_low_precision(
        reason="float32r tiles feeding fp32r matmuls; PSUM accumulation stays fp32"
    ):
        with tc.tile_pool(name="const", bufs=1) as constp:
            cos_sb = constp.tile([P, S], f32)
            sin_sb = constp.tile([P, S], f32)
            mask_sb = constp.tile([P, DT], f32)
            ones_k = constp.tile([P, 1], f32r)
            ones_p = constp.tile([1, P], f32r)
            nc.sync.dma_start(out=cos_sb[:], in_=cosT_d[:])
            nc.sync.dma_start(out=sin_sb[:], in_=sinT_d[:])
            nc.sync.dma_start(out=mask_sb[:], in_=mask_d[:])
            nc.sync.dma_start(out=ones_k[:], in_=ones_k_d[:])
            nc.sync.dma_start(out=ones_p[:], in_=ones_p_d[:])

            # ---- Stage 1: projections (q, k rotary-transposed; v natural) ----
            with (
                tc.tile_pool(name="wpool", bufs=1) as wpool,
                tc.tile_pool(name="xpool", bufs=2) as xpool,
                tc.tile_pool(name="evict", bufs=4) as epool,
                tc.tile_pool(name="ps1", bufs=4, space="PSUM") as ps1,
            ):
                for wdram, odram in ((wq, qT_d), (wk, kT_d)):
                    w_sb = wpool.tile([P, DT * F], f32r, tag="w")
                    for dt in range(DT):
                        nc.sync.dma_start(
                            out=w_sb[:, dt * F : (dt + 1) * F],
                            in_=wdram[dt * P : (dt + 1) * P, :],
                        )
                    for sc in range(4):  # s-chunks of 512
                        x_sb = xpool.tile([P, DT * 512], f32r, tag="x")
                        for dt in range(DT):
                            nc.sync.dma_start(
                                out=x_sb[:, dt * 512 : (dt + 1) * 512],
                                in_=xT[dt * P : (dt + 1) * P, sc * 512 : (sc + 1) * 512],
                            )
                        for ft in range(8):  # feature tiles = heads
                            ps = ps1.tile([P, 512], f32, tag="ps")
                            for dt in range(DT):
                                nc.tensor.matmul(
                                    ps[:],
                                    lhsT=w_sb[:, dt * F + ft * P : dt * F + (ft + 1) * P],
                                    rhs=x_sb[:, dt * 512 : (dt + 1) * 512],
                                    start=(dt == 0),
                                    stop=(dt == DT - 1),
                                )
                            # rotary + eviction
                            o_sb = epool.tile([P, 512], f32r, tag="evq")
                            t2 = epool.tile([P, 512], f32, tag="t2")
                            cs = cos_sb[:, sc * 512 : (sc + 1) * 512]
                            sn = sin_sb[:, sc * 512 : (sc + 1) * 512]
                            nc.vector.tensor_mul(t2[0:64, :], ps[64:128, :], sn[0:64, :])
                            nc.vector.tensor_mul(t2[64:128, :], ps[0:64, :], sn[64:128, :])
                            nc.vector.tensor_mul(o_sb[:], ps[:], cs)
                            nc.vector.tensor_add(o_sb[:], o_sb[:], t2[:])
                            nc.sync.dma_start(
                                out=odram[ft * P : (ft + 1) * P, sc * 512 : (sc + 1) * 512],
                                in_=o_sb[:],
                            )

                # v pass: v[s, f] natural layout
                w_sb = wpool.tile([P, DT * F], f32r, tag="w")
                for dt in range(DT):
                    nc.sync.dma_start(
                        out=w_sb[:, dt * F : (dt + 1) * F],
                        in_=wv[dt * P : (dt + 1) * P, :],
                    )
                for st in range(16):  # s-tiles of 128
                    xv_sb = xpool.tile([P, DT * P], f32r, tag="xv")
                    for dt in range(DT):
                        nc.sync.dma_start(
                            out=xv_sb[:, dt * P : (dt + 1) * P],
                            in_=xT[dt * P : (dt + 1) * P, st * P : (st + 1) * P],
                        )
                    for fc in range(2):  # feature chunks of 512
                        ps = ps1.tile([P, 512], f32, tag="ps")
                        for dt in range(DT):
                            nc.tensor.matmul(
                                ps[:],
                                lhsT=xv_sb[:, dt * P : (dt + 1) * P],
                                rhs=w_sb[:, dt * F + fc * 512 : dt * F + (fc + 1) * 512],
                                start=(dt == 0),
                                stop=(dt == DT - 1),
                            )
                        v_sb = epool.tile([P, 512], f32r, tag="evv")
                        nc.scalar.copy(v_sb[:], ps[:])
                        nc.sync.dma_start(
                            out=v_d[st * P : (st + 1) * P, fc * 512 : (fc + 1) * 512],
                            in_=v_sb[:],
                        )

            # ---- Stage 2: attention per head ----
            with tc.tile_pool(name="attn", bufs=1) as apool:
                attn_sb = []
                for h in range(H_PER_CORE):
                    t = apool.tile([P, S], f32r, tag=f"attn{h}", name=f"attn{h}")
                    attn_sb.append(t)

                with (
                    tc.tile_pool(name="qkv2", bufs=2) as qkvp,
                    tc.tile_pool(name="exp2", bufs=4) as expp,
                    tc.tile_pool(name="small2", bufs=2) as smallp,
                    tc.tile_pool(name="ps_s", bufs=3, space="PSUM") as pss_pool,
                    tc.tile_pool(name="ps_o", bufs=2, space="PSUM") as pso_pool,
                    tc.tile_pool(name="ps_d", bufs=1, space="PSUM") as psd_pool,
                    tc.tile_pool(name="ps_b", bufs=1, space="PSUM") as psb_pool,
                ):
                    for h in range(H_PER_CORE):
                        q_sb = qkvp.tile([P, S], f32r, tag="q")
                        k_sb = qkvp.tile([P, S], f32r, tag="k")
                        v_sb = qkvp.tile([P, S], f32r, tag="v")
                        nc.sync.dma_start(out=q_sb[:], in_=qT_d[h * P : (h + 1) * P, :])
                        nc.sync.dma_start(out=k_sb[:], in_=kT_d[h * P : (h + 1) * P, :])
                        for kt in range(16):
                            nc.sync.dma_start(
                                out=v_sb[:, kt * P : (kt + 1) * P],
                                in_=v_d[kt * P : (kt + 1) * P, h * P : (h + 1) * P],
                            )
                        for qc in range(4):  # q chunks of 512
                            pso = pso_pool.tile([P, 512], f32, tag="pso")
                            psd = psd_pool.tile([P, 512], f32, tag="psd")
                            for kt in range(16):
                                pss = pss_pool.tile([P, 512], f32, tag="pss")
                                nc.tensor.matmul(
                                    pss[:],
                                    lhsT=k_sb[:, kt * P : (kt + 1) * P],
                                    rhs=q_sb[:, qc * 512 : (qc + 1) * 512],
                                    start=True,
                                    stop=True,
                                )
                                e_sb = expp.tile([P, 512], f32r, tag="e")
                                nc.scalar.activation(
                                    e_sb[:],
                                    pss[:],
                                    EXP,
                                    bias=mask_sb[:, kt : kt + 1],
                                    scale=SCALE,
                                )
                                nc.tensor.matmul(
                                    psd[0:1, :],
                                    lhsT=ones_k[:, 0:1],
                                    rhs=e_sb[:],
                                    start=(kt == 0),
                                    stop=(kt == 15),
                                )
                                nc.tensor.matmul(
                                    pso[:],
                                    lhsT=v_sb[:, kt * P : (kt + 1) * P],
                                    rhs=e_sb[:],
                                    start=(kt == 0),
                                    stop=(kt == 15),
                                )
                            r_sb = smallp.tile([1, 512], f32r, tag="r")
                            nc.vector.reciprocal(r_sb[:], psd[0:1, :])
                            psb = psb_pool.tile([P, 512], f32, tag="psb")
                            nc.tensor.matmul(
                                psb[:],
                                lhsT=ones_p[0:1, :],
                                rhs=r_sb[0:1, :],
                                start=True,
                                stop=True,
                            )
                            b_sb = smallp.tile([P, 512], f32, tag="b")
                            nc.scalar.copy(b_sb[:], psb[:])
                            nc.vector.tensor_mul(
                                attn_sb[h][:, qc * 512 : (qc + 1) * 512], pso[:], b_sb[:]
                            )

                # ---- Stage 3: output projection ----
                with (
                    tc.tile_pool(name="wo3", bufs=1) as wop,
                    tc.tile_pool(name="os3", bufs=4) as osp,
                    tc.tile_pool(name="ps3", bufs=4, space="PSUM") as ps3,
                ):
                    wo_sb = wop.tile([P, H_PER_CORE * D], f32r)
                    for h in range(H_PER_CORE):
                        nc.sync.dma_start(
                            out=wo_sb[:, h * D : (h + 1) * D],
                            in_=wo[h * P : (h + 1) * P, :],
                        )
                    for st in range(16):
                        for ec in range(4):
                            ps = ps3.tile([P, 512], f32, tag="ps3")
                            for h in range(H_PER_CORE):
                                nc.tensor.matmul(
                                    ps[:],
                                    lhsT=attn_sb[h][:, st * P : (st + 1) * P],
                                    rhs=wo_sb[:, h * D + ec * 512 : h * D + (ec + 1) * 512],
                                    start=(h == 0),
                                    stop=(h == H_PER_CORE - 1),
                                )
                            o_sb = osp.tile([P, 512], f32, tag="o3")
                            nc.scalar.copy(o_sb[:], ps[:])
                            nc.sync.dma_start(
                                out=out_d[st * P : (st + 1) * P, ec * 512 : (ec + 1) * 512],
                                in_=o_sb[:],
                            )

    nc.compile()
    return nc


def _host_prep(inputs):
    x = np.asarray(inputs["x"], np.float32)
    fc = np.asarray(inputs["freqs_cos"], np.float32)
    fs = np.asarray(inputs["freqs_sin"], np.float32)
    mask = np.asarray(inputs["mask"], np.float32)
    wq = np.asarray(inputs["wq"], np.float32)
    wk = np.asarray(inputs["wk"], np.float32)
    wv = np.asarray(inputs["wv"], np.float32)
    wo = np.asarray(inputs["wo"], np.float32)

    perm = np.concatenate([np.arange(0, HD, 2), np.arange(1, HD, 2)])
    cosT = np.ascontiguousarray(np.concatenate([fc.T, fc.T], 0))
    sinT = np.ascontiguousarray(np.concatenate([-fs.T, fs.T], 0))

    in_maps = []
    for c in range(NCORES):
        b, g = c // 2, c % 2
        colsel = np.concatenate([g * F + h * HD + perm for h in range(H_PER_CORE)])
        in_maps.append(
            {
                "xT": np.ascontiguousarray(x[b].T),
                "wq": np.ascontiguousarray(wq[:, colsel]),
                "wk": np.ascontiguousarray(wk[:, colsel]),
                "wv": np.ascontiguousarray(wv[:, g * F : (g + 1) * F]),
                "wo": np.ascontiguousarray(wo[g * F : (g + 1) * F, :]),
                "cosT": cosT,
                "sinT": sinT,
                "maskT": np.ascontiguousarray(mask[b].reshape(DT, P).T),
                "ones_k": np.ones((P, 1), np.float32),
                "ones_p": np.ones((1, P), np.float32),
            }
        )
    return in_maps


def kernel(**inputs):
    from concourse.bass_utils import run_bass_kernel_spmd

    if "nc" not in _CACHE:
        _CACHE["nc"] = _build()
    nc = _CACHE["nc"]

    in_maps = _host_prep(inputs)
    res = run_bass_kernel_spmd(nc, in_maps, core_ids=list(range(NCORES)))
    out = np.empty((B, S, D), np.float32)
    for b in range(B):
        out[b] = res.results[2 * b]["out"] + res.results[2 * b + 1]["out"]
    return out
